# revision 1
# baseline (speedup 1.0000x reference)
"""Multi-head attention kernel for Trainium2, sharded over 8 NeuronCores.

Problem: B=2, S=2048, D=1024, H=16 heads (DK=64).
  out = softmax(mask ? (XqWq^T)(XkWk^T)^T/8 : -1e9) (XvWv^T) Wo^T

Sharding: core c handles batch b=c//4 and 4 heads hg=c%4 (tensor parallel
over heads, data parallel over batch). Each core computes its partial
output projection y_t = Wo_slice^T-contribution [D, S]; the host sums the
4 partials per batch and transposes back.

On-chip layout is fully "transposed": projections are computed as
Qt = Wq_slice @ Xq^T -> [256, S] with head dim on partitions, so that
scores land as s_T[k, q] (keys on partitions) and the PV matmul consumes
the softmax probabilities directly as the moving operand -- no on-chip
transposes anywhere. Softmax denominators come for free from a ones
column appended to V (row 64 of the PV psum accumulates sum_k p[k,q]).
The mask is folded in multiplicatively after exp (exact: exp(-1e9) == 0
in fp32 and no all-masked rows exist), so no max-subtraction is needed
(scores are O(5), exp cannot overflow).
"""

import sys
import types

sys.path.insert(0, "/opt/trn_rl_repo")

import numpy as np
import ml_dtypes
from contextlib import ExitStack

B, S, D, H = 2, 2048, 1024, 16
DK = D // H  # 64
N_CORES = 8
HPC = H // (N_CORES // B)  # 4 heads per core
EPC = HPC * DK  # 256 head-dims per core
P = 128
BF16 = ml_dtypes.bfloat16

_CACHE = {}


def _patch_tile_drain():
    """This walrus build allows only ONE sync-wait command on a Drain
    (CoreV3GenImpl setupSyncWait). Split the tail-drain waits across
    multiple drain instructions, one wait each."""
    import concourse.tile as tile
    from concourse import mybir
    from concourse.vector_clock import ScopedClock

    if getattr(tile.TileContext, "_drain_split_patch", False):
        return

    def _patched(self, tick_clock, wait_clock):
        nc = self.nc
        drain_inst = nc.sync.drain()
        wait_clock.add_sem_waits(
            drain_inst.ins, ScopedClock({None: tick_clock.global_clock})
        )
        si = drain_inst.ins.sync_info
        if si is not None and si.on_wait is not None and len(si.on_wait) > 1:
            extras = list(si.on_wait[1:])
            del si.on_wait[1:]
            for w in extras:
                d2 = nc.sync.drain()
                d2.ins.sync_info = mybir.SyncInfo(on_wait=[w], on_update=[])
        nc.all_engine_barrier()
        assert self.sems is not None
        popped = nc._tile_sem_poison_stack.pop()
        assert popped is self._sem_poison
        nc.clear_and_free_semaphores(list(self.sems.allocated().values()))
        nc.all_engine_barrier()

    tile.TileContext._drain_and_barrier = _patched
    tile.TileContext._drain_split_patch = True


def _split_multi_waits(nc):
    """This walrus build supports only ONE sync-wait command per
    instruction. Hoist extra waits onto preceding same-engine NoOps --
    engine sequencers process their stream in order, so a NoOp's wait
    gates everything after it on that engine."""
    from concourse import mybir

    ctr = [0]
    for fn in nc.m.functions:
        for blk in fn.blocks:
            insts = blk.instructions
            i = 0
            while i < len(insts):
                inst = insts[i]
                si = getattr(inst, "sync_info", None)
                waits = list(si.on_wait) if si is not None and si.on_wait else []
                if len(waits) > 1:
                    keep = waits[-1]
                    for w in waits[:-1]:
                        ctr[0] += 1
                        nop = mybir.InstNoOp(
                            name=f"wsplit_{ctr[0]}",
                            engine=inst.engine,
                            bass_nofuse=True,
                            sync_info=mybir.SyncInfo(on_wait=[w], on_update=[]),
                        )
                        insts.insert(i, nop)
                        i += 1
                    live = si.on_wait
                    del live[:-1]
                i += 1
    return ctr[0]


def _build_bass():
    import concourse.bass as bass
    import concourse.tile as tile
    from concourse import mybir

    _patch_tile_drain()
    bf16 = mybir.dt.bfloat16
    f32 = mybir.dt.float32
    Exp = mybir.ActivationFunctionType.Exp
    Copy = mybir.ActivationFunctionType.Copy

    nc = bass.Bass()
    xq = nc.dram_tensor("xq_t", [D, S], bf16, kind="ExternalInput")
    xk = nc.dram_tensor("xk_t", [D, S], bf16, kind="ExternalInput")
    xv = nc.dram_tensor("xv_t", [D, S], bf16, kind="ExternalInput")
    mk = nc.dram_tensor("mask_t", [S, S], bf16, kind="ExternalInput")
    wq = nc.dram_tensor("wq_t", [D, EPC], bf16, kind="ExternalInput")
    wk = nc.dram_tensor("wk_t", [D, EPC], bf16, kind="ExternalInput")
    wv = nc.dram_tensor("wv_t", [D, EPC], bf16, kind="ExternalInput")
    wo = nc.dram_tensor("wo_t", [EPC, D], bf16, kind="ExternalInput")
    y = nc.dram_tensor("y_t", [D, S], f32, kind="ExternalOutput")
    # DRAM scratch for partition-broadcasting softmax denominators
    # (SBUF->SBUF DMA cannot broadcast across partitions; DRAM sources can).
    # rsum holds the raw sums row; it is read back reshaped to [128, 8] so
    # the reciprocal uses all DVE lanes (a [1, 1024] reciprocal is ~6.5us,
    # single-lane); rrec holds the reciprocal for the broadcast read.
    rsum_dram = nc.dram_tensor("rsum_scratch", [8, 1024], f32, kind="Internal")
    rrec_dram = nc.dram_tensor("rrec_scratch", [8, 1024], f32, kind="Internal")

    KT = D // P  # 8 contraction tiles for projections
    ST = S // P  # 16 seq tiles
    VW = HPC * (DK + 1)  # 260: V columns + ones column per head

    with tile.TileContext(nc) as tc:
        with ExitStack() as ctx:
            # ---- pools (whole-kernel lifetime) ----
            xt_pool = ctx.enter_context(tc.tile_pool(name="xt", bufs=16))
            w_pool = ctx.enter_context(tc.tile_pool(name="w", bufs=24))
            wo_pool = ctx.enter_context(tc.tile_pool(name="wo", bufs=2))
            mask_pool = ctx.enter_context(tc.tile_pool(name="mask", bufs=ST))
            qt_pool = ctx.enter_context(tc.tile_pool(name="qt", bufs=2))
            kt_pool = ctx.enter_context(tc.tile_pool(name="kt", bufs=2))
            v_pool = ctx.enter_context(tc.tile_pool(name="v", bufs=ST))
            out_pool = ctx.enter_context(tc.tile_pool(name="outsb", bufs=2))
            exp_pool = ctx.enter_context(tc.tile_pool(name="exp", bufs=4))
            p_pool = ctx.enter_context(tc.tile_pool(name="p", bufs=4))
            otmp_pool = ctx.enter_context(tc.tile_pool(name="otmp", bufs=2))
            bc_pool = ctx.enter_context(tc.tile_pool(name="bc", bufs=2))
            r_pool = ctx.enter_context(tc.tile_pool(name="r", bufs=4))
            yev_pool = ctx.enter_context(tc.tile_pool(name="yev", bufs=3))
            ps_s = ctx.enter_context(tc.tile_pool(name="ps_s", bufs=2, space="PSUM"))
            ps_o = ctx.enter_context(tc.tile_pool(name="ps_o", bufs=2, space="PSUM"))

            # ---- weight loads are emitted just-in-time per projection ----
            def load_w(t):
                tiles = []
                for k in range(KT):
                    wt = w_pool.tile([P, EPC], bf16, tag="w", name="w")
                    nc.sync.dma_start(wt[:], t[k * P:(k + 1) * P, :])
                    tiles.append(wt)
                return tiles

            w_sb = {}

            # ---- projections Q/K ----
            # Qt/Kt: [EPC, S] as 2 tiles [128, S]; heads 2m, 2m+1 on
            # partitions [0:64) and [64:128) of tile m. xq and xk are
            # co-resident (xt bufs=16) so the K projection never starves.
            qt_sb = [qt_pool.tile([P, S], bf16, tag="qt", name="qt") for _ in range(2)]
            kt_sb = [kt_pool.tile([P, S], bf16, tag="kt", name="kt") for _ in range(2)]
            for dst_tiles, wname, wdram, x in (
                    (qt_sb, "wq", wq, xq), (kt_sb, "wk", wk, xk)):
                w_sb[wname] = load_w(wdram)
                x_sb = []
                for k in range(KT):
                    xt = xt_pool.tile([P, S], bf16, tag="xt", name="xt")
                    nc.sync.dma_start(xt[:], x[k * P:(k + 1) * P, :])
                    x_sb.append(xt)
                for m in range(2):
                    for q4 in range(4):
                        sl = slice(q4 * 512, (q4 + 1) * 512)
                        ps = ps_s.tile([P, 1024], mybir.dt.float32, tag="ps_s", name="ps_s")
                        for k in range(KT):
                            nc.tensor.matmul(
                                ps[:, 0:512],
                                lhsT=w_sb[wname][k][:, m * P:(m + 1) * P],
                                rhs=x_sb[k][:, sl],
                                start=(k == 0),
                                stop=(k == KT - 1),
                            )
                        nc.scalar.activation(dst_tiles[m][:, sl], ps[:, 0:512], Copy)

            # ---- V projection ----
            # natural layout [S, EPC] as 16 tiles [128, 260] with a ones
            # column appended after each head's 64 dims (row DK of the PV
            # psum then accumulates the softmax denominators for free).
            w_sb["wv"] = load_w(wv)
            wo_sb = []
            for k in range(2):
                wt = wo_pool.tile([P, D], bf16, tag="wo", name="wo")
                nc.sync.dma_start(wt[:], wo[k * P:(k + 1) * P, :])
                wo_sb.append(wt)
            x_sb = []
            for k in range(KT):
                xt = xt_pool.tile([P, S], bf16, tag="xt", name="xt")
                nc.sync.dma_start(xt[:], xv[k * P:(k + 1) * P, :])
                x_sb.append(xt)
            v_sb = []
            for m in range(ST):
                vt = v_pool.tile([P, VW], bf16, tag="v", name="v")
                ps = ps_s.tile([P, 1024], mybir.dt.float32, tag="ps_s", name="ps_s")
                for k in range(KT):
                    nc.tensor.matmul(
                        ps[:, 0:EPC],
                        lhsT=x_sb[k][:, m * P:(m + 1) * P],
                        rhs=w_sb["wv"][k][:, :],
                        start=(k == 0),
                        stop=(k == KT - 1),
                    )
                vs = vt[:].rearrange("p (h x) -> p h x", h=HPC)
                nc.vector.tensor_copy(
                    vs[:, :, 0:DK],
                    ps[:, 0:EPC].rearrange("p (h x) -> p h x", h=HPC),
                )
                nc.vector.memset(vs[:, :, DK:DK + 1], 1.0)
                v_sb.append(vt)

            # ---- attention (qh outer; PV software-pipelined one kt back
            # so the in-order PE queue never waits on exp/mul latency).
            # The output projection for q-half 0 is drip-fed into q-half 1's
            # attention stream (one chunk per kt) so the PE never idles at
            # the phase boundary -- a single >3.4us PE gap re-throttles the
            # HAM clock gate to 1.2GHz and the low duty cycle of cold-clock
            # matmuls keeps it there for the rest of the kernel.
            out_sb = [out_pool.tile([P, S], bf16, tag="outsb", name="outsb") for _ in range(2)]

            def emit_oproj_chunk(m, qc, q0, evict_dve):
                sl = slice(q0 + qc * 512, q0 + (qc + 1) * 512)
                ps = ps_s.tile([P, 1024], mybir.dt.float32, tag="ps_s", name="ps_s")
                for k in range(2):
                    nc.tensor.matmul(
                        ps[:, 0:512],
                        lhsT=wo_sb[k][:, m * P:(m + 1) * P],
                        rhs=out_sb[k][:, sl],
                        start=(k == 0),
                        stop=(k == 1),
                    )
                ev = yev_pool.tile([P, 512], mybir.dt.float32, tag="yev", name="yev")
                if evict_dve:
                    nc.vector.tensor_copy(ev[:], ps[:, 0:512])
                else:
                    nc.scalar.activation(ev[:], ps[:, 0:512], Copy)
                nc.sync.dma_start(y[m * P:(m + 1) * P, sl], ev[:])

            def emit_oproj_tail(q0t):
                for m in range(D // P):
                    for qc in range(2):
                        emit_oproj_chunk(m, qc, q0t, evict_dve=((m + qc) % 2 == 0))

            mask_sb = []
            for k in range(ST):
                mt = mask_pool.tile([P, 1024], bf16, tag="mask", name="mask")
                nc.sync.dma_start(mt[:], mk[k * P:(k + 1) * P, 0:1024])
                mask_sb.append(mt)
            for qh in range(2):
                q0 = qh * 1024
                mask_next = []
                for hp in range(2):
                    po = [
                        ps_o.tile([DK + 1, 1024], mybir.dt.float32, tag="ps_o", name="ps_o")
                        for _ in range(2)
                    ]

                    def emit_pv(kt, pts):
                        for h2 in range(2):
                            h = 2 * hp + h2
                            for qc in range(2):
                                nc.tensor.matmul(
                                    po[h2][:, qc * 512:(qc + 1) * 512],
                                    lhsT=v_sb[kt][:, h * (DK + 1):(h + 1) * (DK + 1)],
                                    rhs=pts[h2][:, qc * 512:(qc + 1) * 512],
                                    start=(kt == 0),
                                    stop=(kt == 15),
                                )

                    prev = None
                    for kt in range(16):
                        psts = []
                        for h2 in range(2):
                            r0 = h2 * DK
                            pss = ps_s.tile([P, 1024], mybir.dt.float32, tag="ps_s", name="ps_s")
                            for qc in range(2):
                                nc.tensor.matmul(
                                    pss[:, qc * 512:(qc + 1) * 512],
                                    lhsT=kt_sb[hp][r0:r0 + DK, kt * P:(kt + 1) * P],
                                    rhs=qt_sb[hp][r0:r0 + DK,
                                                  q0 + qc * 512:q0 + (qc + 1) * 512],
                                    start=True,
                                    stop=True,
                                    tile_position=(r0, 0),
                                )
                            psts.append(pss)
                        pts = []
                        for h2 in range(2):
                            et = exp_pool.tile([P, 1024], bf16, tag="exp", name="exp")
                            nc.scalar.activation(et[:], psts[h2][:], Exp, scale=0.125)
                            pt = p_pool.tile([P, 1024], bf16, tag="p", name="p")
                            nc.vector.tensor_mul(pt[:], et[:], mask_sb[kt][:, 0:1024])
                            pts.append(pt)
                        if qh == 0 and hp == 1:
                            mt = mask_pool.tile([P, 1024], bf16, tag="mask",
                                                name="mask")
                            nc.sync.dma_start(
                                mt[:], mk[kt * P:(kt + 1) * P, 1024:2048])
                            mask_next.append(mt)
                        if prev is not None:
                            emit_pv(kt - 1, prev)
                        prev = pts
                    emit_pv(15, prev)

                    # normalize: row DK of po is sum_k p[k, q]
                    for h2 in range(2):
                        ridx = (qh * 2 + hp) * 2 + h2
                        ot = otmp_pool.tile([DK + 1, 1024], mybir.dt.float32,
                                            tag="otmp", name="otmp")
                        nc.vector.tensor_copy(ot[:], po[h2][:])
                        nc.sync.dma_start(
                            rsum_dram[ridx:ridx + 1, :], ot[DK:DK + 1, :]
                        )
                        rr = r_pool.tile([P, 8], mybir.dt.float32, tag="r",
                                         name="rr")
                        nc.sync.dma_start(
                            rr[:],
                            rsum_dram[ridx:ridx + 1, :].rearrange(
                                "o (p f) -> (o p) f", p=P),
                        )
                        rq = r_pool.tile([P, 8], mybir.dt.float32, tag="r",
                                         name="rq")
                        nc.vector.reciprocal(rq[:], rr[:])
                        nc.sync.dma_start(
                            rrec_dram[ridx:ridx + 1, :].rearrange(
                                "o (p f) -> (o p) f", p=P),
                            rq[:],
                        )
                        bc = bc_pool.tile([DK, 1024], mybir.dt.float32, tag="bc", name="bc")
                        nc.sync.dma_start(
                            bc[:],
                            rrec_dram[ridx:ridx + 1, :].broadcast_to([DK, 1024]),
                        )
                        nc.gpsimd.tensor_mul(
                            out_sb[hp][h2 * DK:(h2 + 1) * DK, q0:q0 + 1024],
                            ot[0:DK, :],
                            bc[:],
                        )
                if qh == 0:
                    mask_sb = mask_next

            # ---- output projection: q-half 0 first (dependency-free by
            # now -- its normalization finished a whole phase ago), which
            # keeps the PE streaming while q-half 1's normalization chain
            # completes ----
            for q0t in (0, 1024):
                emit_oproj_tail(q0t)

    _split_multi_waits(nc)
    return nc


def _get_nc():
    if "nc" not in _CACHE:
        _CACHE["nc"] = _build_bass()
    return _CACHE["nc"]


def kernel(query, key, value, mask, w_q, w_k, w_v, w_o, **unused):
    nc = _get_nc()
    from concourse.bass_utils import run_bass_kernel_spmd

    in_maps = []
    for c in range(N_CORES):
        b = c // (N_CORES // B)
        hg = c % (N_CORES // B)
        e0 = hg * EPC
        in_maps.append({
            "xq_t": np.ascontiguousarray(query[b].T).astype(BF16),
            "xk_t": np.ascontiguousarray(key[b].T).astype(BF16),
            "xv_t": np.ascontiguousarray(value[b].T).astype(BF16),
            "mask_t": np.ascontiguousarray(mask[b].T).astype(BF16),
            "wq_t": np.ascontiguousarray(w_q[e0:e0 + EPC, :].T).astype(BF16),
            "wk_t": np.ascontiguousarray(w_k[e0:e0 + EPC, :].T).astype(BF16),
            "wv_t": np.ascontiguousarray(w_v[e0:e0 + EPC, :].T).astype(BF16),
            "wo_t": np.ascontiguousarray(w_o[:, e0:e0 + EPC].T).astype(BF16),
        })

    res = run_bass_kernel_spmd(nc, in_maps, core_ids=list(range(N_CORES)))
    _CACHE["last_results"] = res

    gpb = N_CORES // B
    out = np.empty((B, S, D), dtype=np.float32)
    for b in range(B):
        acc = res.results[b * gpb]["y_t"].astype(np.float32)
        for c in range(b * gpb + 1, (b + 1) * gpb):
            acc = acc + res.results[c]["y_t"]
        out[b] = acc.T
    return out



# revision 9
# speedup vs baseline: 1.1418x; 1.1418x over previous
"""Multi-head attention kernel for Trainium2, sharded over 8 NeuronCores.

Problem: B=2, S=2048, D=1024, H=16 heads (DK=64).
  out = softmax(mask ? (XqWq^T)(XkWk^T)^T/8 : -1e9) (XvWv^T) Wo^T

Sharding: core c handles batch b=c//4 and 4 heads hg=c%4 (tensor parallel
over heads, data parallel over batch). Each core computes its partial
output projection y_t = Wo_slice^T-contribution [D, S]; the host sums the
4 partials per batch and transposes back.

On-chip layout is fully "transposed": projections are computed as
Qt = Wq_slice @ Xq^T -> [256, S] with head dim on partitions, so that
scores land as s_T[k, q] (keys on partitions) and the PV matmul consumes
the softmax probabilities directly as the moving operand -- no on-chip
transposes anywhere. Softmax denominators come for free from a ones
column appended to V (row 64 of the PV psum accumulates sum_k p[k,q]).

The scalar (ACT) engine is the roofline: 128 exp activations over
[128,1024] psum tiles ~= 147us that no other engine can run. The
schedule keeps ACT on exp only: all psum evictions run on DVE, and the
mask is applied differently per q-half to balance the other engines:
 - q-half 0: multiplicatively after exp (pt = exp * mask) on DVE/GPSIMD
 - q-half 1: additively before exp on the PE (an identity-stationary
   matmul accumulates maskneg = -1e9*(1-mask) into the scores psum;
   exp(0.125*(s-1e9)) == 0 exactly in fp32). This removes the DVE from
   the exp->PV chain and raises PE duty to ~94% so the HAM clock gate
   keeps the PE at 2.4GHz.
V-projection tiles are drip-fed through the scores-psum rotation during
(qh=0, hp=0) -- one [128,256] tile per kt step borrows the psum buffer
freed by that step's h2=0 exp -- so attention starts once K and half of
Q are projected instead of after all projections.
"""

import sys

sys.path.insert(0, "/opt/trn_rl_repo")

import numpy as np
import ml_dtypes
from contextlib import ExitStack

B, S, D, H = 2, 2048, 1024, 16
DK = D // H  # 64
N_CORES = 8
HPC = H // (N_CORES // B)  # 4 heads per core
EPC = HPC * DK  # 256 head-dims per core
P = 128
BF16 = ml_dtypes.bfloat16

_CACHE = {}


def _patch_tile_drain():
    """This walrus build allows only ONE sync-wait command on a Drain
    (CoreV3GenImpl setupSyncWait). Split the tail-drain waits across
    multiple drain instructions, one wait each."""
    import concourse.tile as tile
    from concourse import mybir
    from concourse.vector_clock import ScopedClock

    if getattr(tile.TileContext, "_drain_split_patch", False):
        return

    def _patched(self, tick_clock, wait_clock):
        nc = self.nc
        drain_inst = nc.sync.drain()
        wait_clock.add_sem_waits(
            drain_inst.ins, ScopedClock({None: tick_clock.global_clock})
        )
        si = drain_inst.ins.sync_info
        if si is not None and si.on_wait is not None and len(si.on_wait) > 1:
            extras = list(si.on_wait[1:])
            del si.on_wait[1:]
            for w in extras:
                d2 = nc.sync.drain()
                d2.ins.sync_info = mybir.SyncInfo(on_wait=[w], on_update=[])
        nc.all_engine_barrier()
        assert self.sems is not None
        popped = nc._tile_sem_poison_stack.pop()
        assert popped is self._sem_poison
        nc.clear_and_free_semaphores(list(self.sems.allocated().values()))
        nc.all_engine_barrier()

    tile.TileContext._drain_and_barrier = _patched
    tile.TileContext._drain_split_patch = True


def _split_multi_waits(nc):
    """This walrus build supports only ONE sync-wait command per
    instruction. Hoist extra waits onto preceding same-engine NoOps --
    engine sequencers process their stream in order, so a NoOp's wait
    gates everything after it on that engine."""
    from concourse import mybir

    ctr = [0]
    for fn in nc.m.functions:
        for blk in fn.blocks:
            insts = blk.instructions
            i = 0
            while i < len(insts):
                inst = insts[i]
                si = getattr(inst, "sync_info", None)
                waits = list(si.on_wait) if si is not None and si.on_wait else []
                if len(waits) > 1:
                    keep = waits[-1]
                    for w in waits[:-1]:
                        ctr[0] += 1
                        nop = mybir.InstNoOp(
                            name=f"wsplit_{ctr[0]}",
                            engine=inst.engine,
                            bass_nofuse=True,
                            sync_info=mybir.SyncInfo(on_wait=[w], on_update=[]),
                        )
                        insts.insert(i, nop)
                        i += 1
                    live = si.on_wait
                    del live[:-1]
                i += 1
    return ctr[0]


def _build_bass():
    import concourse.bass as bass
    import concourse.tile as tile
    from concourse import mybir

    _patch_tile_drain()
    bf16 = mybir.dt.bfloat16
    f32 = mybir.dt.float32
    Exp = mybir.ActivationFunctionType.Exp
    Copy = mybir.ActivationFunctionType.Copy

    nc = bass.Bass()
    xq = nc.dram_tensor("xq_t", [D, S], bf16, kind="ExternalInput")
    xk = nc.dram_tensor("xk_t", [D, S], bf16, kind="ExternalInput")
    xv = nc.dram_tensor("xv_t", [D, S], bf16, kind="ExternalInput")
    mk = nc.dram_tensor("mask_t", [S, S // 2], bf16, kind="ExternalInput")
    mn = nc.dram_tensor("maskneg_t", [S, S // 2], bf16, kind="ExternalInput")
    wq = nc.dram_tensor("wq_t", [D, EPC], bf16, kind="ExternalInput")
    wk = nc.dram_tensor("wk_t", [D, EPC], bf16, kind="ExternalInput")
    wv = nc.dram_tensor("wv_t", [D, EPC], bf16, kind="ExternalInput")
    wo = nc.dram_tensor("wo_t", [EPC, D], bf16, kind="ExternalInput")
    ident = nc.dram_tensor("ident_t", [P, P], bf16, kind="ExternalInput")
    y = nc.dram_tensor("y_t", [D, S], f32, kind="ExternalOutput")
    # DRAM scratch for partition-broadcasting softmax denominators
    # (SBUF->SBUF DMA cannot broadcast across partitions; DRAM sources
    # can). rsum holds the raw sums row, read back reshaped to [128, 8]
    # so the reciprocal uses all DVE lanes; rrec holds the reciprocal
    # for the broadcast read.
    rsum_dram = nc.dram_tensor("rsum_scratch", [8, 1024], f32, kind="Internal")
    rrec_dram = nc.dram_tensor("rrec_scratch", [8, 1024], f32, kind="Internal")

    KT = D // P  # 8 contraction tiles for projections
    ST = S // P  # 16 seq tiles
    VW = HPC * (DK + 1)  # 260: V columns + ones column per head

    with tile.TileContext(nc) as tc:
        with ExitStack() as ctx:
            # ---- pools (whole-kernel lifetime) ----
            # one x pool: xk (16 allocs), xq c0 (8), xv (16), xq c1 (8);
            # the rotation makes xv/xq-c1 reuse xk's buffers, which are
            # dead after the K projection.
            x_pool = ctx.enter_context(tc.tile_pool(name="x", bufs=32))
            w_pool = ctx.enter_context(tc.tile_pool(name="w", bufs=24))
            wo_pool = ctx.enter_context(tc.tile_pool(name="wo", bufs=2))
            id_pool = ctx.enter_context(tc.tile_pool(name="id", bufs=1))
            mask_pool = ctx.enter_context(tc.tile_pool(name="mask", bufs=18))
            qt_pool = ctx.enter_context(tc.tile_pool(name="qt", bufs=2))
            kt_pool = ctx.enter_context(tc.tile_pool(name="kt", bufs=2))
            v_pool = ctx.enter_context(tc.tile_pool(name="v", bufs=ST))
            out_pool = ctx.enter_context(tc.tile_pool(name="outsb", bufs=2))
            exp_pool = ctx.enter_context(tc.tile_pool(name="exp", bufs=4))
            p_pool = ctx.enter_context(tc.tile_pool(name="p", bufs=4))
            otmp_pool = ctx.enter_context(tc.tile_pool(name="otmp", bufs=2))
            bc_pool = ctx.enter_context(tc.tile_pool(name="bc", bufs=2))
            r_pool = ctx.enter_context(tc.tile_pool(name="r", bufs=4))
            yev_pool = ctx.enter_context(tc.tile_pool(name="yev", bufs=4))
            dmy_pool = ctx.enter_context(tc.tile_pool(name="dmy", bufs=1))
            ps_s = ctx.enter_context(tc.tile_pool(name="ps_s", bufs=2, space="PSUM"))
            ps_o = ctx.enter_context(tc.tile_pool(name="ps_o", bufs=2, space="PSUM"))

            # ---- dummy exp: pull the ACT table load off the critical path
            dmy = dmy_pool.tile([P, 8], bf16, tag="dmy", name="dmy")
            nc.vector.memset(dmy[:], 0.0)
            nc.scalar.activation(dmy[:], dmy[:], Exp)

            # ---- DMAs. sync queue: wk, xk, xq-c0, then masks/xq-c1
            # interleaved. scalar (ACT) queue is idle in the lead-in:
            # wq/wv/wo/ident and xv.
            w_sb = {}
            for wname, t, q in (("wk", wk, nc.sync), ("wq", wq, nc.scalar),
                                ("wv", wv, nc.scalar)):
                tiles = []
                for k in range(KT):
                    wt = w_pool.tile([P, EPC], bf16, tag="w", name="w")
                    q.dma_start(wt[:], t[k * P:(k + 1) * P, :])
                    tiles.append(wt)
                w_sb[wname] = tiles
            wo_sb = []
            for k in range(2):
                wt = wo_pool.tile([P, D], bf16, tag="wo", name="wo")
                nc.scalar.dma_start(wt[:], wo[k * P:(k + 1) * P, :])
                wo_sb.append(wt)
            id_sb = id_pool.tile([P, P], bf16, tag="id", name="id")
            nc.scalar.dma_start(id_sb[:], ident[:, :])

            # x chunk tiles: [128, 1024] halves of each 128-row band.
            # Only xk and xq-c0 are DMA'd here; xv / masks / xq-c1 are
            # emitted after the lead-in compute so their buffer-reuse
            # waits see the projection reads (pool WAR ordering).
            xk_sb = [[None] * KT for _ in range(2)]
            xq_sb = [[None] * KT for _ in range(2)]
            xv_sb = [[None] * KT for _ in range(2)]
            for k in range(KT):
                for c in range(2):
                    t = x_pool.tile([P, 1024], bf16, tag="x", name="xk")
                    nc.sync.dma_start(t[:], xk[k * P:(k + 1) * P,
                                              c * 1024:(c + 1) * 1024])
                    xk_sb[c][k] = t
            for k in range(KT):
                t = x_pool.tile([P, 1024], bf16, tag="x", name="xq")
                nc.sync.dma_start(t[:], xq[k * P:(k + 1) * P, 0:1024])
                xq_sb[0][k] = t

            # ---- projection helpers ----
            qt_sb = [qt_pool.tile([P, S], bf16, tag="qt", name="qt") for _ in range(2)]
            kt_sb = [kt_pool.tile([P, S], bf16, tag="kt", name="kt") for _ in range(2)]

            def emit_qk_proj(dst_tiles, wname, x_sb, q4, m):
                """One (m, q4) psum group for the Q or K projection."""
                c, half = q4 // 2, (q4 % 2) * 512
                ps = ps_s.tile([P, 1024], f32, tag="ps_s", name="ps_s")
                for k in range(KT):
                    nc.tensor.matmul(
                        ps[:, 0:512],
                        lhsT=w_sb[wname][k][:, m * P:(m + 1) * P],
                        rhs=x_sb[c][k][:, half:half + 512],
                        start=(k == 0),
                        stop=(k == KT - 1),
                    )
                nc.vector.tensor_copy(
                    dst_tiles[m][:, q4 * 512:(q4 + 1) * 512], ps[:, 0:512])

            v_sb = [None] * ST

            def emit_v_proj(m):
                """V tile m: [128 k, 260] with a ones column per head."""
                c, off = m // 8, (m % 8) * P
                vt = v_pool.tile([P, VW], bf16, tag="v", name="v")
                ps = ps_s.tile([P, 1024], f32, tag="ps_s", name="ps_s")
                for k in range(KT):
                    nc.tensor.matmul(
                        ps[:, 0:EPC],
                        lhsT=xv_sb[c][k][:, off:off + P],
                        rhs=w_sb["wv"][k][:, :],
                        start=(k == 0),
                        stop=(k == KT - 1),
                    )
                vs = vt[:].rearrange("p (h x) -> p h x", h=HPC)
                nc.vector.tensor_copy(
                    vs[:, :, 0:DK],
                    ps[:, 0:EPC].rearrange("p (h x) -> p h x", h=HPC),
                )
                nc.vector.memset(vs[:, :, DK:DK + 1], 1.0)
                v_sb[m] = vt

            # ---- lead-in: K proj (all), Q proj q-half 0 ----
            for q4 in range(4):
                for m in range(2):
                    emit_qk_proj(kt_sb, "wk", xk_sb, q4, m)
            for q4 in range(2):
                for m in range(2):
                    emit_qk_proj(qt_sb, "wq", xq_sb, q4, m)

            # ---- deferred DMAs (buffer reuse waits on the reads above)
            for c in range(2):
                for k in range(KT):
                    t = x_pool.tile([P, 1024], bf16, tag="x", name="xv")
                    nc.scalar.dma_start(t[:], xv[k * P:(k + 1) * P,
                                                 c * 1024:(c + 1) * 1024])
                    xv_sb[c][k] = t
            # masks for qh=0 interleaved with xq-c1 on the sync queue
            mask_sb = []
            for kt in range(ST):
                mt = mask_pool.tile([P, 1024], bf16, tag="mask", name="mask")
                nc.sync.dma_start(mt[:], mk[kt * P:(kt + 1) * P, :])
                mask_sb.append(mt)
                if kt % 2 == 0:
                    k = kt // 2
                    t = x_pool.tile([P, 1024], bf16, tag="x", name="xq")
                    nc.sync.dma_start(t[:], xq[k * P:(k + 1) * P, 1024:2048])
                    xq_sb[1][k] = t

            # ---- output projection ----
            out_sb = [out_pool.tile([P, S], bf16, tag="outsb", name="outsb")
                      for _ in range(2)]

            def emit_oproj_chunk(m, qc, q0, evict_act, pool=None):
                sl = slice(q0 + qc * 512, q0 + (qc + 1) * 512)
                if pool is None:
                    pool = ps_s if qc == 0 else ps_o
                ps = pool.tile([P, 512], f32,
                               tag="ps_s" if pool is ps_s else "ps_o",
                               name="ps_op")
                for k in range(2):
                    nc.tensor.matmul(
                        ps[:, 0:512],
                        lhsT=wo_sb[k][:, m * P:(m + 1) * P],
                        rhs=out_sb[k][:, sl],
                        start=(k == 0),
                        stop=(k == 1),
                    )
                ev = yev_pool.tile([P, 512], f32, tag="yev", name="yev")
                if evict_act:
                    nc.scalar.activation(ev[:], ps[:, 0:512], Copy)
                else:
                    nc.vector.tensor_copy(ev[:], ps[:, 0:512])
                nc.sync.dma_start(y[m * P:(m + 1) * P, sl], ev[:])

            # ---- attention ----
            maskneg_sb = [None] * ST

            def attention(qh, hp, inject):
                """One (q-half, head-pair-group) pass. inject: one
                optional callable per kt step, emitted after that step's
                PV so the PE never head-of-line blocks on it."""
                po = [ps_o.tile([DK + 1, 1024], f32, tag="ps_o", name="ps_o")
                      for _ in range(2)]

                def emit_pv(kt, pts):
                    for h2 in range(2):
                        h = 2 * hp + h2
                        for qc in range(2):
                            nc.tensor.matmul(
                                po[h2][:, qc * 512:(qc + 1) * 512],
                                lhsT=v_sb[kt][:, h * (DK + 1):(h + 1) * (DK + 1)],
                                rhs=pts[h2][:, qc * 512:(qc + 1) * 512],
                                start=(kt == 0),
                                stop=(kt == 15),
                            )

                q0 = qh * 1024
                prev = None
                for kt in range(ST):
                    psts = [ps_s.tile([P, 1024], f32, tag="ps_s", name="ps_s")
                            for _ in range(2)]
                    if qh == 1:
                        # additive mask: identity-stationary matmuls seed
                        # the psum with -1e9 on masked elements
                        for h2 in range(2):
                            for qc in range(2):
                                nc.tensor.matmul(
                                    psts[h2][:, qc * 512:(qc + 1) * 512],
                                    lhsT=id_sb[:],
                                    rhs=maskneg_sb[kt][:, qc * 512:(qc + 1) * 512],
                                    start=True,
                                    stop=False,
                                )
                    # scores: qc-outer / h2-inner so the two half-array
                    # matmuls (rows 0-63 vs 64-127) run concurrently
                    for qc in range(2):
                        for h2 in range(2):
                            r0 = h2 * DK
                            nc.tensor.matmul(
                                psts[h2][:, qc * 512:(qc + 1) * 512],
                                lhsT=kt_sb[hp][r0:r0 + DK, kt * P:(kt + 1) * P],
                                rhs=qt_sb[hp][r0:r0 + DK,
                                              q0 + qc * 512:q0 + (qc + 1) * 512],
                                start=(qh == 0),
                                stop=True,
                                tile_position=(r0, 0),
                            )
                    pts = []
                    for h2 in range(2):
                        et = exp_pool.tile([P, 1024], bf16, tag="exp", name="exp")
                        nc.scalar.activation(et[:], psts[h2][:], Exp, scale=0.125)
                        if qh == 0:
                            pt = p_pool.tile([P, 1024], bf16, tag="p", name="p")
                            nc.vector.tensor_mul(pt[:], et[:], mask_sb[kt][:])
                            pts.append(pt)
                        else:
                            pts.append(et)
                    # prefetch next-half (additive) mask during (qh0, hp1)
                    if qh == 0 and hp == 1:
                        mt = mask_pool.tile([P, 1024], bf16, tag="mask",
                                            name="mask")
                        nc.sync.dma_start(mt[:], mn[kt * P:(kt + 1) * P, :])
                        maskneg_sb[kt] = mt
                    if prev is not None:
                        emit_pv(kt - 1, prev)
                    if inject[kt] is not None:
                        inject[kt]()
                    prev = pts
                emit_pv(15, prev)

                # normalize: row DK of po is sum_k p[k, q]
                for h2 in range(2):
                    ridx = (qh * 2 + hp) * 2 + h2
                    ot = otmp_pool.tile([DK + 1, 1024], f32, tag="otmp",
                                        name="otmp")
                    nc.vector.tensor_copy(ot[:], po[h2][:])
                    nc.sync.dma_start(
                        rsum_dram[ridx:ridx + 1, :], ot[DK:DK + 1, :])
                    rr = r_pool.tile([P, 8], f32, tag="r", name="rr")
                    nc.sync.dma_start(
                        rr[:],
                        rsum_dram[ridx:ridx + 1, :].rearrange(
                            "o (p f) -> (o p) f", p=P),
                    )
                    rq = r_pool.tile([P, 8], f32, tag="r", name="rq")
                    nc.vector.reciprocal(rq[:], rr[:])
                    nc.sync.dma_start(
                        rrec_dram[ridx:ridx + 1, :].rearrange(
                            "o (p f) -> (o p) f", p=P),
                        rq[:],
                    )
                    bc = bc_pool.tile([DK, 1024], f32, tag="bc", name="bc")
                    nc.sync.dma_start(
                        bc[:],
                        rrec_dram[ridx:ridx + 1, :].broadcast_to([DK, 1024]),
                    )
                    nc.gpsimd.tensor_mul(
                        out_sb[hp][h2 * DK:(h2 + 1) * DK, q0:q0 + 1024],
                        ot[0:DK, :],
                        bc[:],
                    )

            # phase B: (qh=0, hp=0), V projection injected 1 tile per kt
            attention(0, 0, [lambda m=m: emit_v_proj(m) for m in range(ST)])
            # phase C: (qh=0, hp=1), Q projection q-half 1 injected
            c_inject = [None] * ST
            for i, (q4, m) in enumerate(((2, 0), (2, 1), (3, 0), (3, 1))):
                c_inject[2 + 3 * i] = (
                    lambda q4=q4, m=m: emit_qk_proj(qt_sb, "wq", xq_sb, q4, m))
            attention(0, 1, c_inject)
            # phases E, F: (qh=1, hp=0/1), additive mask. The q-half-0
            # output projection (32 chunks) rides along, one chunk every
            # other kt, through the scores psum rotation.
            ef_chunks = [(m, qc) for m in range(D // P) for qc in range(2)]
            e_inject = [None] * ST
            f_inject = [None] * ST
            for i, (m, qc) in enumerate(ef_chunks[:16]):
                e_inject[i] = (lambda m=m, qc=qc: emit_oproj_chunk(
                    m, qc, 0, evict_act=False, pool=ps_s))
            for i, (m, qc) in enumerate(ef_chunks[16:]):
                f_inject[i] = (lambda m=m, qc=qc: emit_oproj_chunk(
                    m, qc, 0, evict_act=False, pool=ps_s))
            attention(1, 0, e_inject)
            attention(1, 1, f_inject)
            # tail: output projection for q-half 1
            for m in range(D // P):
                for qc in range(2):
                    emit_oproj_chunk(m, qc, 1024, evict_act=((m + qc) % 2 == 0))

    _split_multi_waits(nc)
    return nc


def _get_nc():
    if "nc" not in _CACHE:
        _CACHE["nc"] = _build_bass()
    return _CACHE["nc"]


def kernel(query, key, value, mask, w_q, w_k, w_v, w_o, **unused):
    nc = _get_nc()
    from concourse.bass_utils import run_bass_kernel_spmd

    ident = np.eye(P, dtype=BF16)
    in_maps = []
    for c in range(N_CORES):
        b = c // (N_CORES // B)
        hg = c % (N_CORES // B)
        e0 = hg * EPC
        mt = np.ascontiguousarray(mask[b].T).astype(np.float32)
        in_maps.append({
            "xq_t": np.ascontiguousarray(query[b].T).astype(BF16),
            "xk_t": np.ascontiguousarray(key[b].T).astype(BF16),
            "xv_t": np.ascontiguousarray(value[b].T).astype(BF16),
            "mask_t": np.ascontiguousarray(mt[:, 0:1024]).astype(BF16),
            "maskneg_t": np.ascontiguousarray(
                (mt[:, 1024:2048] - 1.0) * 1e9).astype(BF16),
            "ident_t": ident,
            "wq_t": np.ascontiguousarray(w_q[e0:e0 + EPC, :].T).astype(BF16),
            "wk_t": np.ascontiguousarray(w_k[e0:e0 + EPC, :].T).astype(BF16),
            "wv_t": np.ascontiguousarray(w_v[e0:e0 + EPC, :].T).astype(BF16),
            "wo_t": np.ascontiguousarray(w_o[:, e0:e0 + EPC].T).astype(BF16),
        })

    res = run_bass_kernel_spmd(nc, in_maps, core_ids=list(range(N_CORES)))
    _CACHE["last_results"] = res

    gpb = N_CORES // B
    out = np.empty((B, S, D), dtype=np.float32)
    for b in range(B):
        acc = res.results[b * gpb]["y_t"].astype(np.float32)
        for c in range(b * gpb + 1, (b + 1) * gpb):
            acc = acc + res.results[c]["y_t"]
        out[b] = acc.T
    return out


# revision 16
# speedup vs baseline: 1.2543x; 1.0985x over previous
"""Multi-head attention kernel for Trainium2, sharded over 8 NeuronCores.

Problem: B=2, S=2048, D=1024, H=16 heads (DK=64).
  out = softmax(mask ? (XqWq^T)(XkWk^T)^T/8 : -1e9) (XvWv^T) Wo^T

Sharding: core c handles batch b=c//4 and 4 heads hg=c%4 (tensor parallel
over heads, data parallel over batch). Each core computes its partial
output projection y_t = Wo_slice^T-contribution [D, S]; the host sums the
4 partials per batch and transposes back.

On-chip layout is fully "transposed": projections are computed as
Qt = Wq_slice @ Xq^T -> [256, S] with head dim on partitions, so that
scores land as s_T[k, q] (keys on partitions) and the PV matmul consumes
the softmax probabilities directly as the moving operand -- no on-chip
transposes anywhere. Softmax denominators come for free from a ones
column appended to V (row 64 of the PV psum accumulates sum_k p[k,q]).

The scalar (ACT) engine is the roofline: 128 exp activations over
[128,1024] psum tiles ~= 147us that no other engine can run. The
schedule keeps ACT on exp only: all psum evictions run on DVE, and the
mask is applied differently per q-half to balance the other engines:
 - q-half 0: multiplicatively after exp (pt = exp * mask) on DVE/GPSIMD
 - q-half 1: additively before exp on the PE (an identity-stationary
   matmul accumulates maskneg = -1e9*(1-mask) into the scores psum;
   exp(0.125*(s-1e9)) == 0 exactly in fp32). This removes the DVE from
   the exp->PV chain and raises PE duty to ~94% so the HAM clock gate
   keeps the PE at 2.4GHz.
V-projection tiles are drip-fed through the scores-psum rotation during
(qh=0, hp=0) -- one [128,256] tile per kt step borrows the psum buffer
freed by that step's h2=0 exp -- so attention starts once K and half of
Q are projected instead of after all projections.
"""

import sys

sys.path.insert(0, "/opt/trn_rl_repo")

import numpy as np
import ml_dtypes
from contextlib import ExitStack

B, S, D, H = 2, 2048, 1024, 16
DK = D // H  # 64
N_CORES = 8
HPC = H // (N_CORES // B)  # 4 heads per core
EPC = HPC * DK  # 256 head-dims per core
P = 128
BF16 = ml_dtypes.bfloat16

_CACHE = {}


def _patch_tile_drain():
    """This walrus build allows only ONE sync-wait command on a Drain
    (CoreV3GenImpl setupSyncWait). Split the tail-drain waits across
    multiple drain instructions, one wait each."""
    import concourse.tile as tile
    from concourse import mybir
    from concourse.vector_clock import ScopedClock

    if getattr(tile.TileContext, "_drain_split_patch", False):
        return

    def _patched(self, tick_clock, wait_clock):
        nc = self.nc
        drain_inst = nc.sync.drain()
        wait_clock.add_sem_waits(
            drain_inst.ins, ScopedClock({None: tick_clock.global_clock})
        )
        si = drain_inst.ins.sync_info
        if si is not None and si.on_wait is not None and len(si.on_wait) > 1:
            extras = list(si.on_wait[1:])
            del si.on_wait[1:]
            for w in extras:
                d2 = nc.sync.drain()
                d2.ins.sync_info = mybir.SyncInfo(on_wait=[w], on_update=[])
        nc.all_engine_barrier()
        assert self.sems is not None
        popped = nc._tile_sem_poison_stack.pop()
        assert popped is self._sem_poison
        nc.clear_and_free_semaphores(list(self.sems.allocated().values()))
        nc.all_engine_barrier()

    tile.TileContext._drain_and_barrier = _patched
    tile.TileContext._drain_split_patch = True


def _split_multi_waits(nc):
    """This walrus build supports only ONE sync-wait command per
    instruction. Hoist extra waits onto preceding same-engine NoOps --
    engine sequencers process their stream in order, so a NoOp's wait
    gates everything after it on that engine."""
    from concourse import mybir

    ctr = [0]
    for fn in nc.m.functions:
        for blk in fn.blocks:
            insts = blk.instructions
            i = 0
            while i < len(insts):
                inst = insts[i]
                si = getattr(inst, "sync_info", None)
                waits = list(si.on_wait) if si is not None and si.on_wait else []
                if len(waits) > 1:
                    keep = waits[-1]
                    for w in waits[:-1]:
                        ctr[0] += 1
                        nop = mybir.InstNoOp(
                            name=f"wsplit_{ctr[0]}",
                            engine=inst.engine,
                            bass_nofuse=True,
                            sync_info=mybir.SyncInfo(on_wait=[w], on_update=[]),
                        )
                        insts.insert(i, nop)
                        i += 1
                    live = si.on_wait
                    del live[:-1]
                i += 1
    return ctr[0]


def _build_bass():
    import concourse.bass as bass
    import concourse.tile as tile
    from concourse import mybir

    _patch_tile_drain()
    bf16 = mybir.dt.bfloat16
    f32 = mybir.dt.float32
    Exp = mybir.ActivationFunctionType.Exp
    Copy = mybir.ActivationFunctionType.Copy

    nc = bass.Bass()
    xq = nc.dram_tensor("xq_t", [D, S], bf16, kind="ExternalInput")
    xk = nc.dram_tensor("xk_t", [D, S], bf16, kind="ExternalInput")
    xv = nc.dram_tensor("xv_t", [D, S], bf16, kind="ExternalInput")
    mk = nc.dram_tensor("mask_t", [S, S // 2], bf16, kind="ExternalInput")
    mn = nc.dram_tensor("maskneg_t", [S, S // 2], bf16, kind="ExternalInput")
    wq = nc.dram_tensor("wq_t", [D, EPC], bf16, kind="ExternalInput")
    wk = nc.dram_tensor("wk_t", [D, EPC], bf16, kind="ExternalInput")
    wv = nc.dram_tensor("wv_t", [D, EPC], bf16, kind="ExternalInput")
    wo = nc.dram_tensor("wo_t", [EPC, D], bf16, kind="ExternalInput")
    ident = nc.dram_tensor("ident_t", [P, P], bf16, kind="ExternalInput")
    y = nc.dram_tensor("y_t", [D, S], f32, kind="ExternalOutput")
    # DRAM scratch for partition-broadcasting softmax denominators
    # (SBUF->SBUF DMA cannot broadcast across partitions; DRAM sources
    # can). rsum holds the raw sums row, read back reshaped to [128, 8]
    # so the reciprocal uses all DVE lanes; rrec holds the reciprocal
    # for the broadcast read.
    rsum_dram = nc.dram_tensor("rsum_scratch", [8, 1024], f32, kind="Internal")
    rrec_dram = nc.dram_tensor("rrec_scratch", [8, 1024], f32, kind="Internal")

    KT = D // P  # 8 contraction tiles for projections
    ST = S // P  # 16 seq tiles
    VW = HPC * (DK + 1)  # 260: V columns + ones column per head

    with tile.TileContext(nc) as tc:
        with ExitStack() as ctx:
            # ---- pools (whole-kernel lifetime) ----
            # one x pool: xk (16 allocs), xq c0 (8), xv (16), xq c1 (8);
            # the rotation makes xv/xq-c1 reuse xk's buffers, which are
            # dead after the K projection.
            x_pool = ctx.enter_context(tc.tile_pool(name="x", bufs=32))
            w_pool = ctx.enter_context(tc.tile_pool(name="w", bufs=24))
            wo_pool = ctx.enter_context(tc.tile_pool(name="wo", bufs=2))
            id_pool = ctx.enter_context(tc.tile_pool(name="id", bufs=1))
            mask_pool = ctx.enter_context(tc.tile_pool(name="mask", bufs=18))
            qt_pool = ctx.enter_context(tc.tile_pool(name="qt", bufs=2))
            kt_pool = ctx.enter_context(tc.tile_pool(name="kt", bufs=2))
            v_pool = ctx.enter_context(tc.tile_pool(name="v", bufs=ST))
            out_pool = ctx.enter_context(tc.tile_pool(name="outsb", bufs=2))
            exp_pool = ctx.enter_context(tc.tile_pool(name="exp", bufs=4))
            p_pool = ctx.enter_context(tc.tile_pool(name="p", bufs=4))
            otmp_pool = ctx.enter_context(tc.tile_pool(name="otmp", bufs=2))
            bc_pool = ctx.enter_context(tc.tile_pool(name="bc", bufs=2))
            r_pool = ctx.enter_context(tc.tile_pool(name="r", bufs=4))
            yev_pool = ctx.enter_context(tc.tile_pool(name="yev", bufs=6))
            dmy_pool = ctx.enter_context(tc.tile_pool(name="dmy", bufs=1))
            ps_s = ctx.enter_context(tc.tile_pool(name="ps_s", bufs=2, space="PSUM"))
            ps_o = ctx.enter_context(tc.tile_pool(name="ps_o", bufs=2, space="PSUM"))

            # ---- dummy exp: pull the ACT table load off the critical path
            dmy = dmy_pool.tile([P, 8], bf16, tag="dmy", name="dmy")
            nc.vector.memset(dmy[:], 0.0)
            nc.scalar.activation(dmy[:], dmy[:], Exp)

            # ---- DMAs. Lead-in transfers are split across BOTH HWDGE
            # queues (sync + scalar) for bandwidth; the scalar queue is
            # fully drained before the first exp is emitted, so nothing
            # ever queues in front of an exp.
            w_sb = {}
            for wname, t, q in (("wk", wk, nc.sync), ("wq", wq, nc.scalar),
                                ("wv", wv, nc.scalar)):
                tiles = []
                for k in range(KT):
                    wt = w_pool.tile([P, EPC], bf16, tag="w", name="w")
                    q.dma_start(wt[:], t[k * P:(k + 1) * P, :])
                    tiles.append(wt)
                w_sb[wname] = tiles
            wo_sb = []
            for k in range(2):
                wt = wo_pool.tile([P, D], bf16, tag="wo", name="wo")
                nc.scalar.dma_start(wt[:], wo[k * P:(k + 1) * P, :])
                wo_sb.append(wt)
            id_sb = id_pool.tile([P, P], bf16, tag="id", name="id")
            nc.scalar.dma_start(id_sb[:], ident[:, :])

            # x chunk tiles: [128, 1024] halves of each 128-row band,
            # alternating queues per chunk.
            xk_sb = [[None] * KT for _ in range(2)]
            xq_sb = [[None] * KT for _ in range(2)]
            xv_sb = [[None] * KT for _ in range(2)]

            def load_x(dst, src, c, k, q):
                t = x_pool.tile([P, 1024], bf16, tag="x", name="x")
                q.dma_start(t[:], src[k * P:(k + 1) * P,
                                      c * 1024:(c + 1) * 1024])
                dst[c][k] = t

            for k in range(KT):
                for c in range(2):
                    load_x(xk_sb, xk, c, k,
                           nc.sync if (2 * k + c) % 2 == 0 else nc.scalar)
            for k in range(KT):
                load_x(xq_sb, xq, 0, k,
                       nc.sync if k % 2 == 0 else nc.scalar)

            # ---- projection helpers ----
            qt_sb = [qt_pool.tile([P, S], bf16, tag="qt", name="qt") for _ in range(2)]
            kt_sb = [kt_pool.tile([P, S], bf16, tag="kt", name="kt") for _ in range(2)]

            def emit_qk_proj(dst_tiles, wname, x_sb, q4, m):
                """One (m, q4) psum group for the Q or K projection."""
                c, half = q4 // 2, (q4 % 2) * 512
                ps = ps_s.tile([P, 1024], f32, tag="ps_s", name="ps_s")
                for k in range(KT):
                    nc.tensor.matmul(
                        ps[:, 0:512],
                        lhsT=w_sb[wname][k][:, m * P:(m + 1) * P],
                        rhs=x_sb[c][k][:, half:half + 512],
                        start=(k == 0),
                        stop=(k == KT - 1),
                    )
                nc.vector.tensor_copy(
                    dst_tiles[m][:, q4 * 512:(q4 + 1) * 512], ps[:, 0:512])

            v_sb = [None] * ST

            def emit_v_proj(m):
                """V tile m: [128 k, 260] with a ones column per head."""
                c, off = m // 8, (m % 8) * P
                vt = v_pool.tile([P, VW], bf16, tag="v", name="v")
                ps = ps_s.tile([P, 1024], f32, tag="ps_s", name="ps_s")
                for k in range(KT):
                    nc.tensor.matmul(
                        ps[:, 0:EPC],
                        lhsT=xv_sb[c][k][:, off:off + P],
                        rhs=w_sb["wv"][k][:, :],
                        start=(k == 0),
                        stop=(k == KT - 1),
                    )
                vs = vt[:].rearrange("p (h x) -> p h x", h=HPC)
                nc.vector.tensor_copy(
                    vs[:, :, 0:DK],
                    ps[:, 0:EPC].rearrange("p (h x) -> p h x", h=HPC),
                )
                nc.vector.memset(vs[:, :, DK:DK + 1], 1.0)
                v_sb[m] = vt

            # ---- lead-in: K proj (all), Q proj q-half 0, V proj ----
            for q4 in range(4):
                for m in range(2):
                    emit_qk_proj(kt_sb, "wk", xk_sb, q4, m)
            for q4 in range(2):
                for m in range(2):
                    emit_qk_proj(qt_sb, "wq", xq_sb, q4, m)
            # xv DMAs here so their buffer-reuse waits see the K/Q reads
            for c in range(2):
                for k in range(KT):
                    load_x(xv_sb, xv, c, k,
                           nc.sync if k % 2 == 0 else nc.scalar)
            for m in range(ST):
                emit_v_proj(m)

            # masks for qh=0 interleaved with xq-c1 on the sync queue
            mask_sb = []
            for kt in range(ST):
                mt = mask_pool.tile([P, 1024], bf16, tag="mask", name="mask")
                nc.sync.dma_start(mt[:], mk[kt * P:(kt + 1) * P, :])
                mask_sb.append(mt)
                if kt % 2 == 0:
                    k = kt // 2
                    t = x_pool.tile([P, 1024], bf16, tag="x", name="xq")
                    nc.sync.dma_start(t[:], xq[k * P:(k + 1) * P, 1024:2048])
                    xq_sb[1][k] = t

            # ---- output projection ----
            out_sb = [out_pool.tile([P, S], bf16, tag="outsb", name="outsb")
                      for _ in range(2)]

            def emit_oproj_chunk(m, qc, q0, evict_act, pool=None):
                sl = slice(q0 + qc * 512, q0 + (qc + 1) * 512)
                if pool is None:
                    pool = ps_s if qc == 0 else ps_o
                ps = pool.tile([P, 512], f32,
                               tag="ps_s" if pool is ps_s else "ps_o",
                               name="ps_op")
                for k in range(2):
                    nc.tensor.matmul(
                        ps[:, 0:512],
                        lhsT=wo_sb[k][:, m * P:(m + 1) * P],
                        rhs=out_sb[k][:, sl],
                        start=(k == 0),
                        stop=(k == 1),
                    )
                ev = yev_pool.tile([P, 512], f32, tag="yev", name="yev")
                if evict_act:
                    nc.scalar.activation(ev[:], ps[:, 0:512], Copy)
                    nc.sync.dma_start(y[m * P:(m + 1) * P, sl], ev[:])
                else:
                    nc.vector.tensor_copy(ev[:], ps[:, 0:512])
                    nc.scalar.dma_start(y[m * P:(m + 1) * P, sl], ev[:])

            # ---- attention ----
            maskneg_sb = [None] * ST

            def attention(qh, hp, inject):
                """One (q-half, head-pair-group) pass. inject: one
                optional callable per kt step, emitted after that step's
                PV so the PE never head-of-line blocks on it."""
                po = [ps_o.tile([DK + 1, 1024], f32, tag="ps_o", name="ps_o")
                      for _ in range(2)]

                def emit_pv(kt, pts):
                    for h2 in range(2):
                        h = 2 * hp + h2
                        for qc in range(2):
                            nc.tensor.matmul(
                                po[h2][:, qc * 512:(qc + 1) * 512],
                                lhsT=v_sb[kt][:, h * (DK + 1):(h + 1) * (DK + 1)],
                                rhs=pts[h2][:, qc * 512:(qc + 1) * 512],
                                start=(kt == 0),
                                stop=(kt == 15),
                            )

                q0 = qh * 1024
                prev = None
                for kt in range(ST):
                    psts = [ps_s.tile([P, 1024], f32, tag="ps_s", name="ps_s")
                            for _ in range(2)]
                    if qh == 1:
                        # additive mask: identity-stationary matmuls seed
                        # the psum with -1e9 on masked elements
                        for h2 in range(2):
                            for qc in range(2):
                                nc.tensor.matmul(
                                    psts[h2][:, qc * 512:(qc + 1) * 512],
                                    lhsT=id_sb[:],
                                    rhs=maskneg_sb[kt][:, qc * 512:(qc + 1) * 512],
                                    start=True,
                                    stop=False,
                                )
                    # scores: qc-outer / h2-inner so the two half-array
                    # matmuls (rows 0-63 vs 64-127) run concurrently
                    for qc in range(2):
                        for h2 in range(2):
                            r0 = h2 * DK
                            nc.tensor.matmul(
                                psts[h2][:, qc * 512:(qc + 1) * 512],
                                lhsT=kt_sb[hp][r0:r0 + DK, kt * P:(kt + 1) * P],
                                rhs=qt_sb[hp][r0:r0 + DK,
                                              q0 + qc * 512:q0 + (qc + 1) * 512],
                                start=(qh == 0),
                                stop=True,
                                tile_position=(r0, 0),
                            )
                    pts = []
                    for h2 in range(2):
                        et = exp_pool.tile([P, 1024], bf16, tag="exp", name="exp")
                        nc.scalar.activation(et[:], psts[h2][:], Exp, scale=0.125)
                        if qh == 0:
                            pt = p_pool.tile([P, 1024], bf16, tag="p", name="p")
                            nc.vector.tensor_mul(pt[:], et[:], mask_sb[kt][:])
                            pts.append(pt)
                        else:
                            pts.append(et)
                    # prefetch next-half (additive) mask during (qh0, hp1)
                    if qh == 0 and hp == 1:
                        mt = mask_pool.tile([P, 1024], bf16, tag="mask",
                                            name="mask")
                        nc.sync.dma_start(mt[:], mn[kt * P:(kt + 1) * P, :])
                        maskneg_sb[kt] = mt
                    if prev is not None:
                        emit_pv(kt - 1, prev)
                    if inject[kt] is not None:
                        inject[kt]()
                    prev = pts
                emit_pv(15, prev)

                # normalize: row DK of po is sum_k p[k, q]. The chained
                # DMAs run on the GPSIMD (SWDGE) queue: their round-trip
                # semaphore waits would head-of-line block every later
                # transfer (mask prefetch, y writes) on a HWDGE queue.
                for h2 in range(2):
                    ridx = (qh * 2 + hp) * 2 + h2
                    ot = otmp_pool.tile([DK + 1, 1024], f32, tag="otmp",
                                        name="otmp")
                    nc.vector.tensor_copy(ot[:], po[h2][:])
                    nc.gpsimd.dma_start(
                        rsum_dram[ridx:ridx + 1, :], ot[DK:DK + 1, :])
                    rr = r_pool.tile([P, 8], f32, tag="r", name="rr")
                    nc.gpsimd.dma_start(
                        rr[:],
                        rsum_dram[ridx:ridx + 1, :].rearrange(
                            "o (p f) -> (o p) f", p=P),
                    )
                    rq = r_pool.tile([P, 8], f32, tag="r", name="rq")
                    nc.vector.reciprocal(rq[:], rr[:])
                    nc.gpsimd.dma_start(
                        rrec_dram[ridx:ridx + 1, :].rearrange(
                            "o (p f) -> (o p) f", p=P),
                        rq[:],
                    )
                    bc = bc_pool.tile([DK, 1024], f32, tag="bc", name="bc")
                    nc.gpsimd.dma_start(
                        bc[:],
                        rrec_dram[ridx:ridx + 1, :].broadcast_to([DK, 1024]),
                    )
                    nc.gpsimd.tensor_mul(
                        out_sb[hp][h2 * DK:(h2 + 1) * DK, q0:q0 + 1024],
                        ot[0:DK, :],
                        bc[:],
                    )

            # phase B: (qh=0, hp=0)
            attention(0, 0, [None] * ST)
            # phase C: (qh=0, hp=1), Q projection q-half 1 injected
            c_inject = [None] * ST
            for i, (q4, m) in enumerate(((2, 0), (2, 1), (3, 0), (3, 1))):
                c_inject[3 + 4 * i] = (
                    lambda q4=q4, m=m: emit_qk_proj(qt_sb, "wq", xq_sb, q4, m))
            attention(0, 1, c_inject)
            # phases E, F: (qh=1, hp=0/1), additive mask
            attention(1, 0, [None] * ST)
            attention(1, 1, [None] * ST)
            # tail: all 64 output-projection chunks, dense. q-half 0
            # first (its normalization finished two phases ago) so the
            # PE streams while F's normalization chain completes.
            i = 0
            for q0 in (0, 1024):
                for m in range(D // P):
                    for qc in range(2):
                        emit_oproj_chunk(m, qc, q0,
                                         evict_act=(i % 2 == 0))
                        i += 1

    _split_multi_waits(nc)
    return nc


def _get_nc():
    if "nc" not in _CACHE:
        _CACHE["nc"] = _build_bass()
    return _CACHE["nc"]


def kernel(query, key, value, mask, w_q, w_k, w_v, w_o, **unused):
    nc = _get_nc()
    from concourse.bass_utils import run_bass_kernel_spmd

    ident = np.eye(P, dtype=BF16)
    in_maps = []
    for c in range(N_CORES):
        b = c // (N_CORES // B)
        hg = c % (N_CORES // B)
        e0 = hg * EPC
        mt = np.ascontiguousarray(mask[b].T).astype(np.float32)
        in_maps.append({
            "xq_t": np.ascontiguousarray(query[b].T).astype(BF16),
            "xk_t": np.ascontiguousarray(key[b].T).astype(BF16),
            "xv_t": np.ascontiguousarray(value[b].T).astype(BF16),
            "mask_t": np.ascontiguousarray(mt[:, 0:1024]).astype(BF16),
            "maskneg_t": np.ascontiguousarray(
                (mt[:, 1024:2048] - 1.0) * 1e9).astype(BF16),
            "ident_t": ident,
            "wq_t": np.ascontiguousarray(w_q[e0:e0 + EPC, :].T).astype(BF16),
            "wk_t": np.ascontiguousarray(w_k[e0:e0 + EPC, :].T).astype(BF16),
            "wv_t": np.ascontiguousarray(w_v[e0:e0 + EPC, :].T).astype(BF16),
            "wo_t": np.ascontiguousarray(w_o[:, e0:e0 + EPC].T).astype(BF16),
        })

    res = run_bass_kernel_spmd(nc, in_maps, core_ids=list(range(N_CORES)))
    _CACHE["last_results"] = res

    gpb = N_CORES // B
    out = np.empty((B, S, D), dtype=np.float32)
    for b in range(B):
        acc = res.results[b * gpb]["y_t"].astype(np.float32)
        for c in range(b * gpb + 1, (b + 1) * gpb):
            acc = acc + res.results[c]["y_t"]
        out[b] = acc.T
    return out
